# revision 28
# baseline (speedup 1.0000x reference)
"""Trainium2 Bass kernel for nn_Block_70952859730367 (dense transformer block).

Strategy (8 NeuronCores, SPMD, one launch):
  Phase A  (per core): q/k projections for this core's 2 heads (h=2c, 2c+1)
           over ALL B*T tokens via fp8 DoubleRow matmuls (2-term x hi/lo
           split, W hi), staged to fp8 at natural scale and partition-folded
           (SBUF->SBUF DMA) into a [32, 2, t] DoubleRow layout so the causal
           scores ALSO run as fp8 DR matmuls (half the fp32r cost).
           V computed in [d, t] layout (ap=512 DR matmuls), then PE-transposed
           per 128x128 tile into vsd [token, d] fp8 with an fp8 ones column
           for the softmax denominator.
  Phase B  : causal attention per (b, 512-token block): DR fp8 scoresT, exp
           on ACT straight to fp8, causal mask via width-trimmed fp8
           multiplies alternating DVE/Pool, attnV as fp8 DR over key-chunk
           PAIRS with the ones column giving the denominator.
  A2A x2   : token ownership is split — core c owns tokens [256c, 256c+256)
           of BOTH batches. Stage-0 AllToAll ships b=0 attention outputs as
           soon as the b=0 blocks finish, so the stage-0 FFN (PE-bound)
           overlaps the b=1 attention (ACT/exp-bound). Stage-1 A2A + FFN run
           at the end, with mm2-0 covering the stage-1 collective latency.
  Phase D  (per 256-token stage): proj (W hi) + residual; mm1 2-term
           (x2 hi/lo x W1 hi) + SiLU with an h hi/lo split; mm2 3-term
           (h*W2hi + hlo*W2hi + h*W2lo) + residual. The h split halves the
           dominant fp8 quantization error, paying for the dropped W1 lo
           term, and halves the resident W1 footprint (both D stages reread
           W1/W2, so both must stay resident).

All fp8 is e4m3 with power-of-2 per-tensor weight scaling; hi+lo splits
share one scale so both accumulate in the same PSUM group. Residual stream
kept in full fp32.
"""
import numpy as np
import ml_dtypes

import concourse.bass as bass
import concourse.tile as tile
from concourse import bacc, mybir
from concourse import bass_utils

B, T, C = 2, 2048, 1024
H, HS, FF = 16, 64, 4096
NT = B * T                      # 4096 tokens, b-major
NCORES = 8
TOK = NT // NCORES              # 512 tokens per core (256 from each batch)
HTOK = TOK // 2                 # 256
SCALE = HS ** -0.5              # 0.125

F32 = mybir.dt.float32
FP8 = mybir.dt.float8e4
AF = mybir.ActivationFunctionType
ALU = mybir.AluOpType
DR = mybir.MatmulPerfMode.DoubleRow
E4M3 = ml_dtypes.float8_e4m3

_PROGRAM = None
_PROG_SCALES = None
_NO_OVERLAP = False
_DEBUG_PROJ_ONLY = False
_DEBUG_ATN = False
_DEBUG_AFIN = False
LAST_EXEC_NS = None


def _emit(nc, tc, io, scales, use_collective=True, stop_after=None):
    x8, wqk, wv8, wp8, w18, w28, b1_d, xTown, masks, out_d = (
        io["x8"], io["wqk"], io["wv8"], io["wp8"], io["w18"], io["w28"],
        io["b1"], io["xTown"], io["masks"], io["out"])
    s_q, s_k, s_v, s_p, s_1, s_2 = scales
    exp_scale = float(SCALE)   # q/k stored at natural scale in fp8
    from contextlib import ExitStack
    from concourse import masks as cmasks

    # Pool stack discipline: pools close strictly LIFO, so open order is
    # lifetime order — program-long pools first, attention pools next, the
    # xt pool (closed right after phase A) last.
    outer = ExitStack()
    const = outer.enter_context(tc.tile_pool(name="const", bufs=1))
    wqk_sb = const.tile([128, 4, 2, 2, 128], FP8, tag="wqk")
    wv_sb = const.tile([128, 2, 4, 2, 128], FP8, tag="wv")
    ident = const.tile([128, 128], FP8, tag="ident")
    masks_sb = const.tile([128, 4, 512], FP8, tag="masks")
    b1_sb = const.tile([128, 32, 1], F32, tag="b1")
    wp_sb = const.tile([128, 8, 1, 4, 2, 128], FP8, tag="wp")
    xTown_sb = const.tile([128, 8, 2, HTOK], F32, tag="xTown")

    # W1 hi slabs: all 8 stay resident (both D stages reread them); DMAs
    # emitted later, paced by the driver.
    w1pool = outer.enter_context(tc.tile_pool(name="w1s", bufs=8))
    w1_sl = [w1pool.tile([128, 4, 4, 2, 128], FP8, tag="w1",
                         name=f"w1g{g}") for g in range(8)]

    def emit_w1(g):
        nc.sync.dma_start(out=w1_sl[g], in_=w18.ap()[:, 4 * g:4 * (g + 1)])

    # DRAM bounce tiles for the two collective stages (fp8)
    dram = outer.enter_context(tc.tile_pool(name="dram", bufs=1, space="DRAM"))
    a2a_in = [dram.tile([8, 128, HTOK], FP8, tag=f"a2ai{s}",
                        name=f"a2ai{s}") for s in (0, 1)]
    a2a_out = [dram.tile([8, 128, HTOK], FP8, tag=f"a2ao{s}",
                         name=f"a2ao{s}") for s in (0, 1)]

    # ---------------- pools + input streams ----------------
    qkp = outer.enter_context(tc.tile_pool(name="qkp", bufs=2,
                                           space="PSUM"))
    # phase-D tile pool; bufs=1: stage-1 tiles reuse stage-0 slots (the PE
    # emission order already serializes each slot across stages)
    dp = outer.enter_context(tc.tile_pool(name="dp", bufs=1))
    hfp = outer.enter_context(tc.tile_pool(name="hfp", bufs=4))
    attn_scope = ExitStack()
    qkvpool = attn_scope.enter_context(tc.tile_pool(name="qkv", bufs=1))
    # q/k in fp8 DoubleRow-folded layout: [64 = (h, p<32), 2 = m, 2048 t],
    # head-dim d = m*32 + p, folded by 2 SBUF->SBUF DMAs per (tb, q|k).
    qdr = [qkvpool.tile([64, 2, 2048], FP8, tag=f"q{b}", name=f"q{b}")
           for b in range(2)]
    kdr = [qkvpool.tile([64, 2, 2048], FP8, tag=f"k{b}", name=f"k{b}")
           for b in range(2)]
    vsd = [qkvpool.tile([128, 16, 2, 128], FP8, tag=f"vsd{b}",
                        name=f"vsd{b}") for b in range(2)]
    stgpool = attn_scope.enter_context(tc.tile_pool(name="stg", bufs=3))
    for b in range(2):
        # cols 64..127 static per b: col 64 = 1.0 (softmax denominator via
        # the attnV matmul), cols 65.. = 0 so av rows 65..127 stay finite
        nc.gpsimd.memset(vsd[b][:, :, :, 64:128], 0.0)
        nc.gpsimd.memset(vsd[b][:, :, :, 64:65], 1.0)

    scp = attn_scope.enter_context(tc.tile_pool(name="scp", bufs=2,
                                                space="PSUM"))
    avp = attn_scope.enter_context(tc.tile_pool(name="avp", bufs=2,
                                                space="PSUM"))
    ep = attn_scope.enter_context(tc.tile_pool(name="ep", bufs=22))
    afp = attn_scope.enter_context(tc.tile_pool(name="afp", bufs=2))
    rp = attn_scope.enter_context(tc.tile_pool(name="rp", bufs=1))
    mask_tog = [0]

    xt_scope = ExitStack()
    xtpool = xt_scope.enter_context(tc.tile_pool(name="xt", bufs=6))
    nc.sync.dma_start(out=wqk_sb, in_=wqk.ap())
    nc.sync.dma_start(out=wv_sb, in_=wv8.ap())
    cmasks.make_identity(nc, ident)
    nc.scalar.dma_start(out=b1_sb, in_=b1_d.ap())
    xts = []
    for tb in range(8):
        xt = xtpool.tile([128, 2, 8, 512], FP8, tag="xt", name=f"xt{tb}")
        nc.sync.dma_start(out=xt, in_=x8.ap()[:, tb])
        xts.append(xt)
        if tb == 0:
            nc.scalar.dma_start(out=masks_sb, in_=masks.ap())
        if tb == 2:
            nc.scalar.dma_start(out=wp_sb, in_=wp8.ap()[:, :, 0:1])
        if tb == 4:
            nc.scalar.dma_start(out=xTown_sb, in_=xTown.ap())

    def emit_a_qk(tb):
        b, j = tb // 4, tb % 4
        xt = xts[tb]
        for d in range(2):           # q, k
            ps = qkp.tile([128, 512], F32, tag="qkp",
                          name=f"qk{tb}_{d}")[:]
            nmm = 0
            for xl in range(2):      # x hi + lo, W hi only
                for p in range(4):
                    nmm += 1
                    nc.tensor.matmul(
                        ps[:],
                        lhsT=wqk_sb[:, p, d, :, :],
                        rhs=xt[:, xl, 2 * p:2 * p + 2, :],
                        start=(nmm == 1), stop=(nmm == 8),
                        perf_mode=DR)
            # stage at natural scale in fp8, then partition-fold to the
            # DoubleRow layout [64=(h,p), 2=m, t] (head dim = m*32+p)
            stg = stgpool.tile([128, 512], FP8, tag="qs",
                               name=f"qs{tb}_{d}")
            nc.vector.tensor_scalar_mul(stg[:], ps,
                                        float(1.0 / (s_q, s_k)[d]))
            dst = (qdr, kdr)[d][b]
            for h in range(2):
                for m in range(2):
                    nc.scalar.dma_start(
                        out=dst[32 * h:32 * (h + 1), m,
                                512 * j:512 * (j + 1)],
                        in_=stg[64 * h + 32 * m:64 * h + 32 * m + 32, :])

    def emit_a_v(tb):
        xt = xts[tb]
        terms = ((0, 0), (1, 0), (0, 1))   # (x part, w part)
        ps = qkp.tile([128, 512], F32, tag="qkp", name=f"v{tb}")[:]
        nmm = 0
        for xl, hl in terms:
            for p in range(4):
                nmm += 1
                nc.tensor.matmul(
                    ps[:],
                    lhsT=wv_sb[:, hl, p, :, :],
                    rhs=xt[:, xl, 2 * p:2 * p + 2, :],
                    start=(nmm == 1), stop=(nmm == 12),
                    perf_mode=DR)
        vsb = stgpool.tile([128, 512], FP8, tag="vsb", name=f"vsb{tb}")
        nc.vector.tensor_scalar_mul(vsb[:], ps, float(1.0 / s_v))
        return vsb

    vsbs = {}

    def emit_a_vtrans(tb):
        b, j = tb // 4, tb % 4
        vsb = vsbs.pop(tb)
        # fp8 PE transpose requires PSUM output element step of 2
        tp = qkp.tile([128, 4, 128, 2], FP8, tag="qkp", name=f"tp{tb}")
        for i in range(4):
            nc.tensor.transpose(tp[:, i, :, 0],
                                vsb[:, 128 * i:128 * (i + 1)], ident[:])
        nc.vector.tensor_copy(
            vsd[b][:, 4 * j:4 * (j + 1), :, 0:64],
            tp[:, :, :, 0].rearrange("p s (h q) -> p s h q", h=2))

    def emit_a(tb):
        emit_a_qk(tb)
        vsbs[tb] = emit_a_v(tb)

    pending = []

    def emit_attnv(pend):
        e, h, pr, b, j, av = pend
        npairs = 2 * (j + 1)
        nc.tensor.matmul(
            av[h][:],
            lhsT=vsd[b][:, 2 * pr:2 * pr + 2, h, :],
            rhs=e[:],
            start=(pr == npairs - 1), stop=(pr == 0),
            perf_mode=DR, skip_group_check=True)

    def emit_b_scores(b, j, defer=False):
        if (4 * b + j) in vsbs:
            emit_a_vtrans(4 * b + j)   # own-diagonal vsd chunks
        t0 = 512 * j
        kmax = 4 * (j + 1)
        npairs = kmax // 2
        av = [avp.tile([128, 512], F32, tag="av",
                       name=f"av{b}_{j}_{_h}") for _h in range(2)]
        for pr in range(npairs - 1, -1, -1):   # diag pairs first
            k0, k1 = 2 * pr, 2 * pr + 1
            m0, m1 = k0 - 4 * j, k1 - 4 * j
            for h in range(2):
                sp = scp.tile([128, 2, 512], F32, tag="sc",
                              name=f"sp{b}_{j}_{pr}_{h}")
                for ki, k in enumerate((k0, k1)):
                    m = k - 4 * j
                    lo = 128 * m if m >= 2 else 0   # causal col trim
                    nc.tensor.matmul(
                        sp[:, ki, lo:512],
                        lhsT=kdr[b][32 * h:32 * (h + 1), :,
                                    128 * k:128 * (k + 1)],
                        rhs=qdr[b][32 * h:32 * (h + 1), :,
                                   t0 + lo:t0 + 512],
                        start=True, stop=True, perf_mode=DR,
                        skip_group_check=True)
                e = ep.tile([128, 2, 512], FP8, tag="e")
                if m0 >= 2:
                    # top diagonal pair: exp only the causal-reachable
                    # columns, zero the rest, mask the 128-wide triangle
                    for ki, m in ((0, m0), (1, m1)):
                        nc.scalar.activation(
                            e[:, ki, 128 * m:512], sp[:, ki, 128 * m:512],
                            AF.Exp, scale=exp_scale)
                        nc.gpsimd.memset(e[:, ki, 0:128 * m], 0.0)
                else:
                    nc.scalar.activation(e[:], sp[:], AF.Exp,
                                         scale=exp_scale)
                for ki, m in ((0, m0), (1, m1)):
                    if m >= 0:   # diagonal-block chunk: mask
                        lo = 128 * m if m >= 2 else 0
                        hi = 128 * (m + 1)
                        eng = (nc.vector, nc.gpsimd)[mask_tog[0] % 2]
                        mask_tog[0] += 1
                        eng.tensor_mul(e[:, ki, lo:hi], e[:, ki, lo:hi],
                                       masks_sb[:, m, lo:hi])
                pending.append((e, h, pr, b, j, av))
            if not defer:
                while len(pending) > 4:
                    emit_attnv(pending.pop(0))
        return av

    def emit_b_tail(b, j, av):
        while pending and pending[0][4] == j and pending[0][3] == b:
            emit_attnv(pending.pop(0))
        for h in range(2):
            r = rp.tile([1, 512], F32, tag="r")
            nc.vector.reciprocal(r[:], av[h][64:65, :])
            rb = rp.tile([64, 512], F32, tag="rb")
            nc.gpsimd.partition_broadcast(rb[:], r[:])
            af = afp.tile([64, 512], FP8, tag="af")
            nc.vector.tensor_mul(af[:], av[h][0:64, :], rb[:])
            # block (b, j) tokens split across dest cores 2j (first 256)
            # and 2j+1 (last 256); stage index == b
            for half in range(2):
                nc.sync.dma_start(
                    out=a2a_in[b][2 * j + half, 64 * h:64 * (h + 1), :],
                    in_=af[:, HTOK * half:HTOK * (half + 1)])

    # ---------------- phase D (per 256-token stage) ----------------
    atn = [None, None]
    x2f = [None, None]
    x2q = [None, None]
    x2lo = [None, None]
    h8 = [None, None]
    h8lo = [None, None]

    def emit_c(s):
        atn[s] = dp.tile([128, 8, HTOK], FP8, tag="atn", name=f"atn{s}")
        if use_collective:
            nc.gpsimd.collective_compute(
                "AllToAll", ALU.bypass,
                replica_groups=[list(range(NCORES))],
                ins=[a2a_in[s].opt()], outs=[a2a_out[s].opt()])
            src = a2a_out[s]
        else:  # timing-estimation build: stand-in DMAs, same byte volume
            src = a2a_in[s]
        for qt in range(4):
            nc.sync.dma_start(
                out=atn[s][:, 2 * qt:2 * qt + 2, :],
                in_=src[2 * qt:2 * qt + 2].rearrange("s p t -> p s t"))

    def emit_d_proj(s):
        x2f[s] = dp.tile([128, 8, HTOK], F32, tag="x2f", name=f"x2f{s}")
        x2q[s] = dp.tile([128, 8, HTOK], FP8, tag="x2q", name=f"x2q{s}")
        x2lo[s] = dp.tile([128, 8, HTOK], FP8, tag="x2lo", name=f"x2lo{s}")
        for cc in range(8):
            ps = qkp.tile([128, HTOK], F32, tag="qkp", name=f"pj{s}_{cc}")
            for p in range(4):
                nc.tensor.matmul(
                    ps[:], lhsT=wp_sb[:, cc, 0, p, :, :],
                    rhs=atn[s][:, 2 * p:2 * p + 2, :],
                    start=(p == 0), stop=(p == 3), perf_mode=DR)
            nc.vector.scalar_tensor_tensor(
                out=x2f[s][:, cc, :], in0=ps[:], scalar=float(1.0 / s_p),
                in1=xTown_sb[:, cc, s, :], op0=ALU.mult, op1=ALU.add)
            nc.gpsimd.tensor_copy(x2q[s][:, cc, :], x2f[s][:, cc, :])
            nc.vector.scalar_tensor_tensor(
                out=x2lo[s][:, cc, :], in0=x2q[s][:, cc, :], scalar=-1.0,
                in1=x2f[s][:, cc, :], op0=ALU.mult, op1=ALU.add)

    def emit_d_mm1(s):
        h8[s] = dp.tile([128, 32, HTOK], FP8, tag="h8", name=f"h8{s}")
        h8lo[s] = dp.tile([128, 32, HTOK], FP8, tag="h8lo", name=f"h8lo{s}")
        for fc in range(32):
            wt = w1_sl[fc // 4]
            fi = fc % 4
            ps = qkp.tile([128, HTOK], F32, tag="qkp", name=f"m1_{s}_{fc}")
            nmm = 0
            for rhs_t in (x2q[s], x2lo[s]):   # W hi only
                for p in range(4):
                    nmm += 1
                    nc.tensor.matmul(
                        ps[:], lhsT=wt[:, fi, p, :, :],
                        rhs=rhs_t[:, 2 * p:2 * p + 2, :],
                        start=(nmm == 1), stop=(nmm == 8), perf_mode=DR)
            # SiLU to f32, then h hi/lo fp8 split (mm2 is 3-term h-split)
            hf = hfp.tile([128, HTOK], F32, tag="hf",
                          name=f"hf{s}_{fc}")
            nc.scalar.activation(hf[:], ps[:], AF.Silu,
                                 scale=float(1.0 / s_1),
                                 bias=b1_sb[:, fc, :])
            nc.gpsimd.tensor_copy(h8[s][:, fc, :], hf[:])
            nc.vector.scalar_tensor_tensor(
                out=h8lo[s][:, fc, :], in0=h8[s][:, fc, :], scalar=-1.0,
                in1=hf[:], op0=ALU.mult, op1=ALU.add)

    def emit_d_mm2(s, w2_sl):
        for cc in range(8):
            ps = qkp.tile([128, HTOK], F32, tag="qkp", name=f"m2_{s}_{cc}")
            nmm = 0
            for ht, hl in ((h8[s], 0), (h8lo[s], 0), (h8[s], 1)):
                for pf in range(16):
                    nmm += 1
                    nc.tensor.matmul(
                        ps[:], lhsT=w2_sl[cc][hl][:, pf, :, :],
                        rhs=ht[:, 2 * pf:2 * pf + 2, :],
                        start=(nmm == 1), stop=(nmm == 48), perf_mode=DR)
            ot = dp.tile([128, HTOK], F32, tag="out", name=f"ot{s}_{cc}",
                         bufs=2)
            nc.vector.scalar_tensor_tensor(
                out=ot[:], in0=ps[:], scalar=float(1.0 / s_2),
                in1=x2f[s][:, cc, :], op0=ALU.mult, op1=ALU.add)
            nc.sync.dma_start(
                out=out_d.ap()[128 * cc:128 * (cc + 1), s, :], in_=ot[:])

    # ---------------- driver ----------------
    if stop_after == "a":
        for tb in range(8):
            emit_a(tb)
            emit_a_vtrans(tb)
        xt_scope.close()
        attn_scope.close()
        outer.close()
        return

    emit_a(0)
    emit_a(1)
    av00 = emit_b_scores(0, 0)
    emit_a(2)
    emit_w1(0)
    av01 = emit_b_scores(0, 1)
    emit_b_tail(0, 0, av00)
    emit_a(3)
    emit_w1(1)
    av02 = emit_b_scores(0, 2)
    emit_b_tail(0, 1, av01)
    vsbs[4] = emit_a_v(4)   # v first: copy overlaps qk4's matmuls
    emit_a_qk(4)
    emit_w1(2)
    av03 = emit_b_scores(0, 3)
    emit_b_tail(0, 2, av02)
    emit_a(5)
    emit_w1(3)
    av10 = emit_b_scores(1, 0)
    emit_b_tail(0, 3, av03)
    emit_c(0)               # stage-0 A2A: b=0 attention is complete
    emit_a(6)
    emit_w1(4)
    av11 = emit_b_scores(1, 1)
    emit_b_tail(1, 0, av10)
    emit_a(7)
    emit_w1(5)
    xt_scope.close()        # x chunks fully consumed
    av12 = emit_b_scores(1, 2)
    emit_b_tail(1, 1, av11)
    emit_w1(6)
    emit_w1(7)
    if _NO_OVERLAP:
        av13 = emit_b_scores(1, 3)
        emit_b_tail(1, 2, av12)
        emit_b_tail(1, 3, av13)
    else:
        emit_d_proj(0)      # stage-0 FFN fills the b=1 exp-bound window
        av13 = emit_b_scores(1, 3, defer=True)
        emit_b_tail(1, 2, av12)
        emit_d_mm1(0)
        emit_b_tail(1, 3, av13)

    if stop_after == "b":
        attn_scope.close()
        outer.close()
        return

    # free attention pools before the W2-resident phase
    attn_scope.close()

    if _DEBUG_AFIN:
        afd = dp.tile([128, 8, HTOK], FP8, tag="atn", name="afd")
        for qt in range(4):
            nc.sync.dma_start(
                out=afd[:, 2 * qt:2 * qt + 2, :],
                in_=a2a_in[0][2 * qt:2 * qt + 2].rearrange("s p t -> p s t"))
        for p in range(8):
            tmp = dp.tile([128, HTOK], F32, tag="out", name=f"afd{p}",
                          bufs=2)
            nc.vector.tensor_copy(tmp[:], afd[:, p, :])
            nc.sync.dma_start(out=out_d.ap()[128 * p:128 * (p + 1), 0, :],
                              in_=tmp[:])
            nc.sync.dma_start(out=out_d.ap()[128 * p:128 * (p + 1), 1, :],
                              in_=tmp[:])
        outer.close()
        return
    if _DEBUG_ATN:
        tmp = dp.tile([128, HTOK], F32, tag="out", name="atndump", bufs=2)
        for p in range(8):
            tmp = dp.tile([128, HTOK], F32, tag="out", name=f"atnd{p}",
                          bufs=2)
            nc.vector.tensor_copy(tmp[:], atn[0][:, p, :])
            nc.sync.dma_start(out=out_d.ap()[128 * p:128 * (p + 1), 0, :],
                              in_=tmp[:])
            nc.sync.dma_start(out=out_d.ap()[128 * p:128 * (p + 1), 1, :],
                              in_=tmp[:])
        outer.close()
        return

    if _NO_OVERLAP:
        emit_d_proj(0)
        emit_d_mm1(0)
    emit_c(1)               # stage-1 A2A; mm2-0 covers its latency
    w2pool = outer.enter_context(tc.tile_pool(name="w2s", bufs=16))
    w2_sl = []
    for cc in range(8):
        halves = []
        for hl in range(2):
            w2t_ = w2pool.tile([128, 16, 2, 128], FP8, tag="w2",
                               name=f"w2c{cc}_{hl}")
            nc.scalar.dma_start(out=w2t_, in_=w28.ap()[:, cc, hl])
            halves.append(w2t_)
        w2_sl.append(halves)
    if stop_after == "c":
        outer.close()
        return
    if _DEBUG_PROJ_ONLY:
        for cc in range(8):
            nc.sync.dma_start(
                out=out_d.ap()[128 * cc:128 * (cc + 1), 0, :],
                in_=x2f[0][:, cc, :])
        emit_d_proj(1)
        for cc in range(8):
            nc.sync.dma_start(
                out=out_d.ap()[128 * cc:128 * (cc + 1), 1, :],
                in_=x2f[1][:, cc, :])
    else:
        emit_d_mm2(0, w2_sl)
        emit_d_proj(1)
        emit_d_mm1(1)
        emit_d_mm2(1, w2_sl)

    outer.close()


def build(scales=None, single_core=False, stop_after=None, repeats=1):
    global _PROGRAM, _PROG_SCALES
    if scales is None:
        scales = _PROG_SCALES or (2048.0,) * 6
    if (not single_core and repeats == 1 and _PROGRAM is not None
            and _PROG_SCALES == tuple(scales)):
        return _PROGRAM
    nc = bacc.Bacc("TRN2", target_bir_lowering=False, debug=False,
                   num_devices=1 if single_core else NCORES)
    io = {
        "x8": nc.dram_tensor("x8", [128, 8, 2, 8, 512], FP8,
                             kind="ExternalInput"),
        "wqk": nc.dram_tensor("wqk", [128, 4, 2, 2, 128], FP8,
                              kind="ExternalInput"),
        "wv8": nc.dram_tensor("wv8", [128, 2, 4, 2, 128], FP8,
                              kind="ExternalInput"),
        "wp8": nc.dram_tensor("wp8", [128, 8, 2, 4, 2, 128], FP8,
                              kind="ExternalInput"),
        "w18": nc.dram_tensor("w18", [128, 32, 4, 2, 128], FP8,
                              kind="ExternalInput"),
        "w28": nc.dram_tensor("w28", [128, 8, 2, 16, 2, 128], FP8,
                              kind="ExternalInput"),
        "b1": nc.dram_tensor("b1", [128, 32, 1], F32, kind="ExternalInput"),
        "xTown": nc.dram_tensor("xTown", [128, 8, 2, HTOK], F32,
                                kind="ExternalInput"),
        "masks": nc.dram_tensor("masks", [128, 4, 512], FP8,
                                kind="ExternalInput"),
        "out": nc.dram_tensor("out", [C, 2, HTOK], F32,
                              kind="ExternalOutput"),
    }
    with tile.TileContext(nc) as tc:
        for _r in range(repeats):
            _emit(nc, tc, io, scales, use_collective=not single_core,
                  stop_after=stop_after)
    nc.compile()
    if single_core or repeats != 1:
        return nc
    _PROGRAM = nc
    _PROG_SCALES = tuple(scales)
    return nc


def _pow2_scale(w, target=192.0):
    m = float(np.abs(w).max())
    return float(2.0 ** np.floor(np.log2(target / max(m, 1e-30))))


def _q8(a):
    return np.asarray(a, np.float32).astype(E4M3)


def _hilo(w, s):
    """hi/lo fp8 split of w*s, both parts at the same scale s."""
    ws = np.asarray(w, np.float32) * s
    hi = _q8(ws)
    lo = _q8(ws - hi.astype(np.float32))
    return hi, lo


def _pack_lhst(w, s, ncol_grp, pairs):
    """w [K, M] -> [128, ncol_grp, 2(hl), pairs, 2, 128] fp8 (lhsT DR layout).

    out[p_, g, hl, p, m, col] = (hi|lo)[(2p+m)*128 + p_, g*128 + col]
    """
    K, M = w.shape
    assert K == pairs * 256 and M == ncol_grp * 128
    hi, lo = _hilo(w, s)

    def lay(a):
        # [K, M] -> [pairs, 2, 128, ncol_grp, 128] -> [128, ncol_grp, pairs, 2, 128]
        a = a.reshape(pairs, 2, 128, ncol_grp, 128)
        return a.transpose(2, 3, 0, 1, 4)
    out = np.stack([lay(hi), lay(lo)], axis=2)  # [128, g, hl, pairs, 2, 128]
    return np.ascontiguousarray(out.transpose(0, 1, 2, 3, 4, 5))


def kernel(x, Wq, Wk, Wv, Wproj, W1, b1, W2):
    global LAST_EXEC_NS
    x = np.asarray(x, np.float32)
    xT = np.ascontiguousarray(x.reshape(NT, C).T)          # [C, NT]
    Wq = np.asarray(Wq, np.float32)
    Wk = np.asarray(Wk, np.float32)
    Wv = np.asarray(Wv, np.float32)
    WprojT = np.asarray(Wproj, np.float32).T               # [d 1024, c 1024]
    W1t = np.asarray(W1, np.float32).T                     # [C, FF]
    W2t = np.asarray(W2, np.float32).T                     # [FF, C]
    b1v = np.asarray(b1, np.float32).reshape(FF, 1)

    s_q = _pow2_scale(Wq)
    s_k = _pow2_scale(Wk)
    s_v = _pow2_scale(Wv)
    s_p = _pow2_scale(WprojT)
    s_1 = _pow2_scale(W1t)
    s_2 = _pow2_scale(W2t)
    scales = (s_q, s_k, s_v, s_p, s_1, s_2)

    # x8: [128, tb, hl, kc, 512] hi/lo fp8 split (same scale)
    xhi = _q8(xT)
    xlo = _q8(xT - xhi.astype(np.float32))
    x8 = np.stack([a.reshape(8, 128, 8, 512).transpose(1, 2, 0, 3)
                   for a in (xhi, xlo)], axis=2)
    x8 = np.ascontiguousarray(x8)
    # shared weights
    wp8 = _pack_lhst(WprojT, s_p, 8, 4)
    w18 = np.ascontiguousarray(_pack_lhst(W1t, s_1, 32, 4)[:, :, 0])
    w28 = _pack_lhst(W2t, s_2, 8, 16)
    b1_h = np.ascontiguousarray(
        b1v.reshape(32, 128, 1).transpose(1, 0, 2))

    s_i = np.arange(128)[:, None, None]
    m_i = np.arange(4)[None, :, None]
    t_i = np.arange(512)[None, None, :]
    masks = _q8((128 * m_i + s_i <= t_i).astype(np.float32))

    in_maps = []
    for c in range(NCORES):
        h0, h1 = 2 * c, 2 * c + 1
        qcols = np.concatenate([Wq[h0], Wq[h1]], axis=1)   # [C, 128]
        kcols = np.concatenate([Wk[h0], Wk[h1]], axis=1)
        vcols = np.concatenate([Wv[h0], Wv[h1]], axis=1)
        # wqk: [128, p, d, m, col] — W hi part only (2-term x-split)
        pq = _pack_lhst(qcols, s_q, 1, 4)  # [128, 1, 2, 4, 2, 128]
        pk = _pack_lhst(kcols, s_k, 1, 4)
        wqk_c = np.stack([pq[:, 0, 0], pk[:, 0, 0]], axis=2)
        pv = _pack_lhst(vcols, s_v, 1, 4)[:, 0]         # [128, hl, p, 2, 128]
        # xTown: [128, cc, stage, 256] — core c owns tokens [256c, 256c+256)
        # of each batch
        xtown_c = np.stack(
            [xT[:, 256 * c:256 * c + HTOK],
             xT[:, T + 256 * c:T + 256 * c + HTOK]], axis=1)
        xtown_c = np.ascontiguousarray(
            xtown_c.reshape(8, 128, 2, HTOK).transpose(1, 0, 2, 3))
        in_maps.append({
            "x8": x8,
            "wqk": np.ascontiguousarray(wqk_c),
            "wv8": np.ascontiguousarray(pv),
            "wp8": wp8, "w18": w18, "w28": w28, "b1": b1_h,
            "xTown": xtown_c,
            "masks": masks,
        })

    nc = build(scales)
    res = bass_utils.run_bass_kernel_spmd(
        nc, in_maps, core_ids=list(range(NCORES)))

    full = np.empty((NT, C), np.float32)
    for c in range(NCORES):
        o = res.results[c]["out"]          # [C, 2, 256]
        full[256 * c:256 * c + HTOK, :] = o[:, 0, :].T
        full[T + 256 * c:T + 256 * c + HTOK, :] = o[:, 1, :].T
    return full.reshape(B, T, C)
